# revision 1
# baseline (speedup 1.0000x reference)
"""Trainium2 Bass kernel for ActivationPNASimplifiedLayer (GNN message passing).

Strategy (8 NeuronCores, SPMD, full inputs in / full output out):
  * Host (index-only preprocessing): degree-sort nodes, deal them round-robin
    to the 8 cores (so all cores share one compiled per-tile slot schedule),
    and build all gather index tables.
  * Random 256-byte row movement on TRN2 is HBM-latency-bound per SDMA
    engine; dma_gather on 4 SWDGE queues is the fastest primitive (~3.5
    ns/row) but takes int16 indices only.  Two-level gather per round
    (<=32768 staging rows):
      1. class gathers: hn = h*norm lives in four 25000-row quarter tables;
         four compact dma_gathers fetch each round's messages (grouped by
         src quarter) and a contiguous DMA restages them into a DRAM
         staging buffer.
      2. regather: dma_gather per tile-group reads staging (int16-safe local
         rows) into padded node-major layout [128 nodes, W_t, 64]; slot 0 is
         the node's own hn row, pad slots repeat the first edge (exact for
         max/min, subtracted analytically for sum/sumsq).
  * Segment sum/sumsq/max/min are free-axis tensor_reduce ops; PNA scalers,
    13-way mean, ReLU are batched wide elementwise math; BatchNorm stats use
    a ones-matmul partition reduction + a 512-byte AllReduce over 8 cores.
  * Host reassembles the permuted per-core outputs (pure index scatter).
"""

import math
import os

import numpy as np

# ---------------------------------------------------------------- constants
N_NODES = 100000
N_EDGES = 1200000
FEAT = 64
P = 128
NCORES = 8
NQ = 4                      # src classes (quarter tables, int16-indexable)
QROWS = N_NODES // NQ       # 25000
AVG_D_LOG = float(np.log(13.0))
EPS_STD = 1e-5
EPS_BN = 1e-5

SRMAX = 32640               # max staging rows per round
GBUD = 64                   # max regather-group slot width
NQUEUES = 4                 # SWDGE queues (latency-wall concurrency)
L16 = 16

_CACHE = {}
LAST_RESULTS = None


def _wrap16(vals):
    """Index list -> dma_gather int16 layout [128, n/16] (wrapped, replicated)."""
    v = np.asarray(vals, np.int16)
    n = len(v)
    assert n % L16 == 0
    blk = v.reshape(n // L16, L16).T
    return np.tile(blk, (NCORES, 1))


# ---------------------------------------------------------------- host side
def _build_schedule(src, dst):
    C = NCORES
    src = np.asarray(src, dtype=np.int64)
    dst = np.asarray(dst, dtype=np.int64)
    deg = np.bincount(dst, minlength=N_NODES).astype(np.int64)
    row_start = np.zeros(N_NODES + 1, np.int64)
    np.cumsum(deg, out=row_start[1:])
    order = np.argsort(-deg, kind="stable")
    deg_sorted = deg[order]

    npc = N_NODES // C
    nt = math.ceil(npc / P)                                    # 98
    D_t = [int(deg_sorted[min(t * P * C, N_NODES - 1)]) for t in range(nt)]
    W_t = [d + 1 for d in D_t]

    t_idx = np.arange(nt)
    p_idx = np.arange(P)
    rank_pt = t_idx[None, :] * (P * C) + p_idx[:, None] * C

    cores = []
    for c in range(C):
        ranks = rank_pt + c
        vmask = ranks < N_NODES
        ids = np.where(vmask, order[np.minimum(ranks, N_NODES - 1)], 0)
        dg = np.where(vmask, deg[ids], 0)
        st = row_start[ids]
        cores.append((ids, dg, st, vmask))

    # entry counts per (core, tile, class): own + edges
    n_ptq = np.zeros((C, nt, NQ), np.int64)
    for c in range(C):
        ids, dg, st, _ = cores[c]
        for t in range(nt):
            n_ptq[c, t] += np.bincount(ids[:, t] // QROWS, minlength=NQ)
            for p in range(P):
                d = int(dg[p, t])
                if d:
                    n_ptq[c, t] += np.bincount(
                        src[st[p, t]:st[p, t] + d] // QROWS, minlength=NQ)

    # rounds: consecutive tiles whose common staging layout fits SRMAX
    rounds = []
    tlo = 0
    while tlo < nt:
        thi = tlo
        cols = None
        while thi < nt:
            cand = n_ptq[:, tlo:thi + 1, :].sum(axis=1)
            ccols = np.ceil(cand.max(axis=0) / P).astype(np.int64)
            if int(ccols.sum()) * P > SRMAX and thi > tlo:
                break
            cols = ccols
            thi += 1
        rounds.append((tlo, thi, [int(x) for x in cols]))
        tlo = thi
    rt_max = max(thi - tlo for tlo, thi, _ in rounds)

    offs = np.zeros(nt + 1, np.int64)
    np.cumsum(W_t, out=offs[1:])
    groups = []
    for ri, (rlo, rhi, _) in enumerate(rounds):
        glo = rlo
        cur = 0
        for t in range(rlo, rhi):
            if cur + W_t[t] > GBUD and cur > 0:
                groups.append((ri, glo, t))
                glo, cur = t, 0
            cur += W_t[t]
        groups.append((ri, glo, rhi))
    gb_max = max(int(offs[thi] - offs[tlo]) for _, tlo, thi in groups)

    per_core = []
    for c in range(C):
        ids, dg, st, vmask = cores[c]
        cidx_parts = []
        ridx_parts = []
        for ri, (rlo, rhi, cols) in enumerate(rounds):
            lists = [[] for _ in range(NQ)]
            cb = np.zeros(NQ + 1, np.int64)
            np.cumsum(np.asarray(cols, np.int64) * P, out=cb[1:])
            # staging row per entry: class block + p-major packing
            rrow = np.zeros((rhi - rlo, P, max(W_t[rlo:rhi])), np.int64)
            for t in range(rlo, rhi):
                for p in range(P):
                    d = int(dg[p, t])
                    ent = [int(ids[p, t])]
                    if d:
                        ent.extend(src[st[p, t]:st[p, t] + d].tolist())
                    for s, u in enumerate(ent):
                        q = u // QROWS
                        j = len(lists[q])
                        lists[q].append(u - q * QROWS)
                        rrow[t - rlo, p, s] = cb[q] + (j % P) * cols[q] + j // P
            for q in range(NQ):
                want = cols[q] * P
                assert len(lists[q]) <= want
                lists[q].extend([0] * (want - len(lists[q])))
                cidx_parts.append(_wrap16(lists[q]))
            for (gri, glo, ghi) in groups:
                if gri != ri:
                    continue
                sw = int(offs[ghi] - offs[glo])
                rl = np.empty((sw, P), np.int64)
                col = 0
                for t in range(glo, ghi):
                    d_all = dg[:, t]
                    for s in range(W_t[t]):
                        ss = np.where(s <= d_all, s, np.where(d_all >= 1, 1, 0))
                        rl[col] = rrow[t - rlo, np.arange(P), ss]
                        col += 1
                ridx_parts.append(_wrap16(rl.reshape(-1)))
        cidx = np.concatenate(cidx_parts, axis=1).astype(np.int16)
        ridx = np.concatenate(ridx_parts, axis=1).astype(np.int16)
        cpad = np.asarray(D_t)[None, :] - dg
        meta = np.concatenate([dg, vmask.astype(np.int64), cpad],
                              axis=1).astype(np.int32)
        per_core.append(dict(cidx=cidx, ridx=ridx, meta=meta,
                             ids=ids, vmask=vmask))

    # per-round column extents in the concatenated cidx/ridx arrays
    cro = []
    rro = []
    cpos = 0
    rpos = 0
    for ri, (rlo, rhi, cols) in enumerate(rounds):
        clen = sum(cq * P // L16 for cq in cols)
        rlen = 0
        for (gri, glo, ghi) in groups:
            if gri == ri:
                rlen += int(offs[ghi] - offs[glo]) * P // L16
        cro.append((cpos, clen))
        rro.append((rpos, rlen))
        cpos += clen
        rpos += rlen
    return dict(nt=nt, W_t=W_t, offs=[int(x) for x in offs], rounds=rounds,
                groups=groups, rt_max=rt_max, gb_max=gb_max,
                per_core=per_core, cro=cro, rro=rro,
                cidx_cols=per_core[0]["cidx"].shape[1],
                ridx_cols=per_core[0]["ridx"].shape[1])


# -------------------------------------------------------------- device side
def _mid_bcast(ap2d, n):
    import concourse.bass as bass
    return bass.AP(tensor=ap2d.tensor, offset=ap2d.offset,
                   ap=[ap2d.ap[0], [0, n], ap2d.ap[1]])


def _build_program(sched):
    import concourse.bass as bass
    import concourse.tile as tile
    from concourse import bacc, mybir

    f32 = mybir.dt.float32
    i32 = mybir.dt.int32
    i16 = mybir.dt.int16
    Alu = mybir.AluOpType
    Act = mybir.ActivationFunctionType
    AxX = mybir.AxisListType.X

    nt = sched["nt"]
    W_t = sched["W_t"]
    offs = sched["offs"]
    rounds = sched["rounds"]
    groups = sched["groups"]
    rt_max = sched["rt_max"]
    gb_max = sched["gb_max"]

    nc = bacc.Bacc("TRN2", target_bir_lowering=False, debug=False,
                   num_devices=NCORES, num_swdge_queues=NQUEUES)

    h_d = nc.dram_tensor("h_in", [N_NODES, FEAT], f32, kind="ExternalInput").ap()
    norm_d = nc.dram_tensor("norm_in", [N_NODES, 1], f32,
                            kind="ExternalInput").ap()
    bnw_d = nc.dram_tensor("bnw_in", [FEAT], f32, kind="ExternalInput").ap()
    bnb_d = nc.dram_tensor("bnb_in", [FEAT], f32, kind="ExternalInput").ap()
    cidx_d = nc.dram_tensor("cidx_in", [P, sched["cidx_cols"]], i16,
                            kind="ExternalInput").ap()
    ridx_d = nc.dram_tensor("ridx_in", [P, sched["ridx_cols"]], i16,
                            kind="ExternalInput").ap()
    meta_d = nc.dram_tensor("meta_in", [P, 3 * nt], i32,
                            kind="ExternalInput").ap()
    nown_d = nc.dram_tensor("nown_in", [P, nt], f32, kind="ExternalInput").ap()
    out_d = nc.dram_tensor("out", [P, nt * FEAT], f32,
                           kind="ExternalOutput").ap()

    qcount = [0]

    def nextq():
        qcount[0] += 1
        return qcount[0] % NQUEUES

    with tile.TileContext(nc) as tc:
        with (
            tc.tile_pool(name="dram", bufs=1, space="DRAM") as dpool,
            tc.tile_pool(name="stagp", bufs=3, space="DRAM") as stpoold,
            tc.tile_pool(name="shdram", bufs=1, space="DRAM") as shpool,
            tc.tile_pool(name="const", bufs=1) as cpool,
            tc.tile_pool(name="scal", bufs=1) as spool,
        ):
            hnq = [dpool.tile([QROWS, FEAT], f32, name=f"hnq{q}", tag=f"hnq{q}")
                   for q in range(NQ)]
            cc_in = dpool.tile([1, 2 * FEAT], f32)
            cc_out = shpool.tile([1, 2 * FEAT], f32, addr_space="Shared")

            # ---- index/meta loads + per-node scalars ----
            meta_sb = cpool.tile([P, 3 * nt], i32)
            nc.sync.dma_start(out=meta_sb, in_=meta_d)
            meta_f = cpool.tile([P, 3 * nt], f32)
            nc.vector.tensor_copy(out=meta_f, in_=meta_sb)
            nown = cpool.tile([P, nt], f32)
            nc.sync.dma_start(out=nown, in_=nown_d)
            degf = meta_f[:, 0:nt]
            valid = meta_f[:, nt:2 * nt]
            cpad = meta_f[:, 2 * nt:3 * nt]

            eps_std = spool.tile([P, 1], f32)
            nc.vector.memset(eps_std, EPS_STD)
            eps_bn = spool.tile([P, 1], f32)
            nc.vector.memset(eps_bn, EPS_BN)

            ds = spool.tile([P, nt], f32)
            nc.vector.tensor_scalar_max(out=ds, in0=degf, scalar1=1.0)
            rdeg = spool.tile([P, nt], f32)
            nc.vector.reciprocal(out=rdeg, in_=ds)
            hb = spool.tile([P, nt], f32)
            nc.vector.tensor_scalar(out=hb, in0=degf, scalar1=0.0,
                                    scalar2=None, op0=Alu.is_gt)
            logd = spool.tile([P, nt], f32)
            nc.scalar.activation(out=logd, in_=degf, func=Act.Ln,
                                 bias=1.0, scale=1.0)
            lsafe = spool.tile([P, nt], f32)
            nc.vector.tensor_scalar_max(out=lsafe, in0=logd, scalar1=0.5)
            rlog = spool.tile([P, nt], f32)
            nc.vector.reciprocal(out=rlog, in_=lsafe)
            t1 = spool.tile([P, nt], f32)
            nc.vector.tensor_scalar(out=t1, in0=logd, scalar1=1.0 / AVG_D_LOG,
                                    scalar2=1.0, op0=Alu.mult, op1=Alu.add)
            t2 = spool.tile([P, nt], f32)
            nc.vector.tensor_scalar_mul(out=t2, in0=rlog, scalar1=AVG_D_LOG)
            sS = spool.tile([P, nt], f32)
            nc.vector.tensor_tensor(out=sS, in0=t1, in1=t2, op=Alu.add)
            t3 = spool.tile([P, nt], f32)
            nc.vector.tensor_tensor(out=t3, in0=nown, in1=sS, op=Alu.mult)
            t4 = spool.tile([P, nt], f32)
            nc.vector.tensor_tensor(out=t4, in0=t3, in1=hb, op=Alu.mult)
            tpre = spool.tile([P, nt], f32)
            nc.vector.tensor_scalar_mul(out=tpre, in0=t4, scalar1=1.0 / 13.0)
            c1 = spool.tile([P, nt], f32)
            nc.vector.tensor_scalar_mul(out=c1, in0=valid, scalar1=1.0 / 13.0)

            # ---- hn quarter tables ----
            BRQ = QROWS // P            # 195
            QTAIL = QROWS - BRQ * P     # 40
            CHR = 65
            with tc.tile_pool(name="hnb", bufs=3) as hpool:
                for q in range(NQ):
                    base = q * QROWS
                    done = 0
                    while done < BRQ:
                        rows = min(CHR, BRQ - done)
                        hch = hpool.tile([P, CHR * FEAT], f32, tag="hch",
                                         name=f"hch{q}_{done}")
                        nch = hpool.tile([P, CHR], f32, tag="nch",
                                         name=f"nch{q}_{done}")
                        nc.sync.dma_start(
                            out=hch[:, :rows * FEAT],
                            in_=bass.AP(tensor=h_d.tensor,
                                        offset=(base + done) * FEAT,
                                        ap=[[BRQ * FEAT, P], [1, rows * FEAT]]))
                        nc.sync.dma_start(
                            out=nch[:, :rows],
                            in_=bass.AP(tensor=norm_d.tensor, offset=base + done,
                                        ap=[[BRQ, P], [1, rows]]))
                        hnch = hpool.tile([P, CHR * FEAT], f32, tag="hnch",
                                          name=f"hnch{q}_{done}")
                        nc.vector.tensor_tensor(
                            out=hnch[:, :rows * FEAT].rearrange(
                                "p (r f) -> p r f", f=FEAT),
                            in0=hch[:, :rows * FEAT].rearrange(
                                "p (r f) -> p r f", f=FEAT),
                            in1=nch[:, :rows].to_broadcast([P, rows, FEAT]),
                            op=Alu.mult)
                        nc.sync.dma_start(
                            out=bass.AP(tensor=hnq[q].tensor,
                                        offset=done * FEAT,
                                        ap=[[BRQ * FEAT, P], [1, rows * FEAT]]),
                            in_=hnch[:, :rows * FEAT])
                        done += rows
                    if QTAIL == 0:
                        continue
                    ht = hpool.tile([P, FEAT], f32, tag="ht", name=f"ht{q}")
                    nc.sync.dma_start(out=ht[:QTAIL, :],
                                      in_=h_d[base + BRQ * P:base + QROWS, :])
                    ntl = hpool.tile([P, 1], f32, tag="ntl", name=f"ntl{q}")
                    nc.sync.dma_start(out=ntl[:QTAIL, :],
                                      in_=norm_d[base + BRQ * P:base + QROWS, :])
                    hnt = hpool.tile([P, FEAT], f32, tag="hnt", name=f"hnt{q}")
                    nc.vector.tensor_tensor(
                        out=hnt[:QTAIL, :], in0=ht[:QTAIL, :],
                        in1=ntl[:QTAIL, :1].to_broadcast([QTAIL, FEAT]),
                        op=Alu.mult)
                    nc.sync.dma_start(out=hnq[q][BRQ * P:QROWS, :],
                                      in_=hnt[:QTAIL, :])

            # ---- main loop ----
            sqw = max(W_t) - 1
            cgc_max = max(max(cols) for _, _, cols in rounds)
            cro = sched["cro"]
            rro = sched["rro"]
            cmax = max(x[1] for x in cro)
            rmax = max(x[1] for x in rro)
            with (
                tc.tile_pool(name="idxp", bufs=2) as idxpool,
                tc.tile_pool(name="cgpool", bufs=2) as cgpool,
                tc.tile_pool(name="gpool", bufs=3) as gpool,
                tc.tile_pool(name="sqpool", bufs=2) as sqpool,
                tc.tile_pool(name="stats", bufs=1) as stpool,
                tc.tile_pool(name="bm", bufs=4) as bmpool,
                tc.tile_pool(name="outr", bufs=2) as orpool,
                tc.tile_pool(name="bnp", bufs=1) as bnpool,
                tc.tile_pool(name="psum", bufs=2, space="PSUM") as pspool,
            ):
                out_pre = dpool.tile([P, nt * FEAT], f32, name="out_pre")
                rs1 = bnpool.tile([P, FEAT], f32)
                rs2 = bnpool.tile([P, FEAT], f32)
                nc.vector.memset(rs1, 0.0)
                nc.vector.memset(rs2, 0.0)

                gi = 0
                for ri, (rlo, rhi, cols) in enumerate(rounds):
                    stag = stpoold.tile([SRMAX, FEAT], f32, tag="stag",
                                        name=f"stag{ri}")
                    cidx_sb = idxpool.tile([P, cmax], i16, tag="cidx",
                                           name=f"cidx{ri}")
                    nc.sync.dma_start(
                        out=cidx_sb[:, :cro[ri][1]],
                        in_=cidx_d[:, cro[ri][0]:cro[ri][0] + cro[ri][1]])
                    ridx_sb = idxpool.tile([P, rmax], i16, tag="ridx",
                                           name=f"ridx{ri}")
                    nc.sync.dma_start(
                        out=ridx_sb[:, :rro[ri][1]],
                        in_=ridx_d[:, rro[ri][0]:rro[ri][0] + rro[ri][1]])
                    cbase = 0
                    rbase = 0
                    cb = 0
                    for q in range(NQ):
                        ncols = cols[q]
                        if ncols == 0:
                            continue
                        ni = ncols * P
                        CG = cgpool.tile([P, cgc_max * FEAT], f32, tag="CG",
                                         name=f"CG{ri}_{q}")
                        nc.gpsimd.dma_gather(
                            out_ap=CG[:, :ncols * FEAT].rearrange(
                                "p (c f) -> p c f", f=FEAT),
                            in_ap=hnq[q],
                            idxs_ap=cidx_sb[:, cbase:cbase + ni // L16],
                            num_idxs=ni, num_idxs_reg=ni, elem_size=FEAT,
                            single_packet=False, queue_num=nextq())
                        cbase += ni // L16
                        nc.sync.dma_start(
                            out=bass.AP(tensor=stag.tensor,
                                        offset=stag.offset + cb * FEAT,
                                        ap=[[ncols * FEAT, P],
                                            [1, ncols * FEAT]]),
                            in_=CG[:, :ncols * FEAT])
                        cb += ncols * P

                    # super-group stats buffers (one round = one super-group)
                    sgn = rhi - rlo
                    s1w = stpool.tile([P, rt_max * FEAT], f32, tag="s1w",
                                      name=f"s1w{ri}")
                    s2w = stpool.tile([P, rt_max * FEAT], f32, tag="s2w",
                                      name=f"s2w{ri}")
                    mxw = stpool.tile([P, rt_max * FEAT], f32, tag="mxw",
                                      name=f"mxw{ri}")
                    mnw = stpool.tile([P, rt_max * FEAT], f32, tag="mnw",
                                      name=f"mnw{ri}")
                    ownw = stpool.tile([P, rt_max * FEAT], f32, tag="ownw",
                                       name=f"ownw{ri}")
                    m1w = stpool.tile([P, rt_max * FEAT], f32, tag="m1w",
                                      name=f"m1w{ri}")

                    rows_used = cb
                    while gi < len(groups) and groups[gi][0] == ri:
                        _, glo, ghi = groups[gi]
                        gi += 1
                        sw = int(offs[ghi] - offs[glo])
                        G = gpool.tile([P, gb_max * FEAT], f32, tag="G",
                                       name=f"G{glo}")
                        nc.gpsimd.dma_gather(
                            out_ap=G[:, :sw * FEAT].rearrange(
                                "p (c f) -> p c f", f=FEAT),
                            in_ap=stag[:rows_used, :],
                            idxs_ap=ridx_sb[:, rbase:rbase + sw * P // L16],
                            num_idxs=sw * P, num_idxs_reg=sw * P,
                            elem_size=FEAT, single_packet=False,
                            queue_num=nextq())
                        rbase += sw * P // L16
                        G3 = G.rearrange("p (w f) -> p w f", f=FEAT)
                        for t in range(glo, ghi):
                            off = int(offs[t] - offs[glo])
                            D = W_t[t] - 1
                            col = (t - rlo) * FEAT
                            edges_t = G3[:, off + 1:off + 1 + D, :] \
                                .rearrange("p w f -> p f w")
                            nc.vector.tensor_reduce(
                                out=s1w[:, col:col + FEAT], in_=edges_t,
                                axis=AxX, op=Alu.add)
                            nc.vector.tensor_reduce(
                                out=mxw[:, col:col + FEAT], in_=edges_t,
                                axis=AxX, op=Alu.max)
                            nc.vector.tensor_reduce(
                                out=mnw[:, col:col + FEAT], in_=edges_t,
                                axis=AxX, op=Alu.min)
                            sq = sqpool.tile([P, sqw * FEAT], f32, tag="sq",
                                             name=f"sq{t}")
                            nc.scalar.activation(
                                out=sq[:, :D * FEAT],
                                in_=G[:, (off + 1) * FEAT:(off + 1 + D) * FEAT],
                                func=Act.Square)
                            nc.vector.tensor_reduce(
                                out=s2w[:, col:col + FEAT],
                                in_=sq[:, :D * FEAT].rearrange(
                                    "p (w f) -> p f w", f=FEAT),
                                axis=AxX, op=Alu.add)
                            nc.vector.tensor_copy(
                                out=ownw[:, col:col + FEAT],
                                in_=G[:, off * FEAT:(off + 1) * FEAT])
                            nc.vector.tensor_copy(
                                out=m1w[:, col:col + FEAT],
                                in_=G[:, (off + 1) * FEAT:(off + 2) * FEAT])

                    # ---- batched per-node math for this round ----
                    nf = sgn * FEAT
                    r3 = lambda ap: ap[:, :nf].rearrange("p (t f) -> p t f",
                                                         f=FEAT)
                    bc = lambda ap: ap[:, rlo:rhi].to_broadcast([P, sgn, FEAT])
                    _bmn = [0]

                    def bm():
                        _bmn[0] += 1
                        return bmpool.tile([P, rt_max * FEAT], f32, tag="bm",
                                           name=f"bm_{ri}_{_bmn[0]}")

                    tA = bm()
                    nc.vector.tensor_tensor(out=r3(tA), in0=r3(m1w),
                                            in1=bc(cpad), op=Alu.mult)
                    s1t = bm()
                    nc.vector.tensor_tensor(out=s1t[:, :nf], in0=s1w[:, :nf],
                                            in1=tA[:, :nf], op=Alu.subtract)
                    mean = bm()
                    nc.vector.tensor_tensor(out=r3(mean), in0=r3(s1t),
                                            in1=bc(rdeg), op=Alu.mult)
                    sqm1 = bm()
                    nc.scalar.activation(out=sqm1[:, :nf], in_=m1w[:, :nf],
                                         func=Act.Square)
                    tB = bm()
                    nc.vector.tensor_tensor(out=r3(tB), in0=r3(sqm1),
                                            in1=bc(cpad), op=Alu.mult)
                    s2t = bm()
                    nc.vector.tensor_tensor(out=s2t[:, :nf], in0=s2w[:, :nf],
                                            in1=tB[:, :nf], op=Alu.subtract)
                    e2 = bm()
                    nc.vector.tensor_tensor(out=r3(e2), in0=r3(s2t),
                                            in1=bc(rdeg), op=Alu.mult)
                    msq = bm()
                    nc.scalar.activation(out=msq[:, :nf], in_=mean[:, :nf],
                                         func=Act.Square)
                    varr = bm()
                    nc.vector.tensor_tensor(out=varr[:, :nf], in0=e2[:, :nf],
                                            in1=msq[:, :nf], op=Alu.subtract)
                    var0 = bm()
                    nc.vector.tensor_scalar_max(out=var0[:, :nf],
                                                in0=varr[:, :nf], scalar1=0.0)
                    stdv = bm()
                    nc.scalar.activation(out=stdv[:, :nf], in_=var0[:, :nf],
                                         func=Act.Sqrt, bias=eps_std, scale=1.0)
                    pre1 = bm()
                    nc.vector.tensor_tensor(out=pre1[:, :nf], in0=mean[:, :nf],
                                            in1=mxw[:, :nf], op=Alu.add)
                    pre2 = bm()
                    nc.vector.tensor_tensor(out=pre2[:, :nf], in0=mnw[:, :nf],
                                            in1=stdv[:, :nf], op=Alu.add)
                    pre = bm()
                    nc.vector.tensor_tensor(out=pre[:, :nf], in0=pre1[:, :nf],
                                            in1=pre2[:, :nf], op=Alu.add)
                    v2 = bm()
                    nc.vector.tensor_tensor(out=r3(v2), in0=r3(pre),
                                            in1=bc(tpre), op=Alu.mult)
                    v1 = bm()
                    nc.vector.tensor_tensor(out=r3(v1), in0=r3(ownw),
                                            in1=bc(c1), op=Alu.mult)
                    outp = bm()
                    nc.vector.tensor_tensor(out=outp[:, :nf], in0=v1[:, :nf],
                                            in1=v2[:, :nf], op=Alu.add)
                    out_r = orpool.tile([P, rt_max * FEAT], f32, tag="outr",
                                        name=f"outr{ri}")
                    nc.scalar.activation(out=out_r[:, :nf],
                                         in_=outp[:, :nf], func=Act.Relu)
                    nc.sync.dma_start(
                        out=out_pre[:, rlo * FEAT:rhi * FEAT],
                        in_=out_r[:, :nf])

                    # BN partial sums
                    orv = out_r[:, :nf].rearrange("p (t f) -> p f t", f=FEAT)
                    rst = bm()
                    nc.vector.tensor_reduce(out=rst[:, :FEAT], in_=orv,
                                            axis=AxX, op=Alu.add)
                    nc.vector.tensor_tensor(out=rs1, in0=rs1,
                                            in1=rst[:, :FEAT], op=Alu.add)
                    sqr = bm()
                    nc.scalar.activation(out=sqr[:, :nf],
                                         in_=out_r[:, :nf],
                                         func=Act.Square)
                    rst2 = bm()
                    nc.vector.tensor_reduce(
                        out=rst2[:, :FEAT],
                        in_=sqr[:, :nf].rearrange("p (t f) -> p f t", f=FEAT),
                        axis=AxX, op=Alu.add)
                    nc.vector.tensor_tensor(out=rs2, in0=rs2,
                                            in1=rst2[:, :FEAT], op=Alu.add)

                # ---- BatchNorm ----
                ones = bnpool.tile([P, 1], f32)
                nc.vector.memset(ones, 1.0)
                rsboth = bnpool.tile([P, 2 * FEAT], f32)
                nc.vector.tensor_copy(out=rsboth[:, :FEAT], in_=rs1)
                nc.vector.tensor_copy(out=rsboth[:, FEAT:], in_=rs2)
                ps = pspool.tile([P, 2 * FEAT], f32, tag="ps")
                nc.tensor.matmul(out=ps[:1, :], lhsT=ones, rhs=rsboth,
                                 start=True, stop=True)
                stats_sb = bnpool.tile([P, 2 * FEAT], f32)
                nc.vector.tensor_copy(out=stats_sb[:1, :], in_=ps[:1, :])
                nc.sync.dma_start(out=cc_in, in_=stats_sb[:1, :])
                nc.gpsimd.collective_compute(
                    "AllReduce", mybir.AluOpType.add,
                    replica_groups=[list(range(NCORES))],
                    ins=[cc_in.opt()], outs=[cc_out.opt()])
                gl = bnpool.tile([P, 2 * FEAT], f32)
                nc.sync.dma_start(out=gl[:1, :], in_=cc_out)

                mu = bnpool.tile([P, FEAT], f32)
                nc.vector.tensor_scalar_mul(out=mu[:1, :], in0=gl[:1, :FEAT],
                                            scalar1=1.0 / N_NODES)
                em2 = bnpool.tile([P, FEAT], f32)
                nc.vector.tensor_scalar_mul(out=em2[:1, :], in0=gl[:1, FEAT:],
                                            scalar1=1.0 / N_NODES)
                musq = bnpool.tile([P, FEAT], f32)
                nc.scalar.activation(out=musq[:1, :], in_=mu[:1, :],
                                     func=Act.Square)
                varb = bnpool.tile([P, FEAT], f32)
                nc.vector.tensor_tensor(out=varb[:1, :], in0=em2[:1, :],
                                        in1=musq[:1, :], op=Alu.subtract)
                stdb = bnpool.tile([P, FEAT], f32)
                nc.scalar.activation(out=stdb[:1, :], in_=varb[:1, :],
                                     func=Act.Sqrt, bias=eps_bn[:1], scale=1.0)
                rstd = bnpool.tile([P, FEAT], f32)
                nc.vector.reciprocal(out=rstd[:1, :], in_=stdb[:1, :])
                bnw_sb = bnpool.tile([P, FEAT], f32)
                nc.sync.dma_start(out=bnw_sb[:1, :], in_=bnw_d[None, :])
                bnb_sb = bnpool.tile([P, FEAT], f32)
                nc.sync.dma_start(out=bnb_sb[:1, :], in_=bnb_d[None, :])
                scsh = bnpool.tile([P, 2 * FEAT], f32)
                nc.vector.tensor_tensor(out=scsh[:1, :FEAT], in0=rstd[:1, :],
                                        in1=bnw_sb[:1, :], op=Alu.mult)
                msc = bnpool.tile([P, FEAT], f32)
                nc.vector.tensor_tensor(out=msc[:1, :], in0=mu[:1, :],
                                        in1=scsh[:1, :FEAT], op=Alu.mult)
                nc.vector.tensor_tensor(out=scsh[:1, FEAT:], in0=bnb_sb[:1, :],
                                        in1=msc[:1, :], op=Alu.subtract)
                onesr = bnpool.tile([P, P], f32)
                nc.vector.memset(onesr[:1, :], 1.0)
                psb = pspool.tile([P, 2 * FEAT], f32, tag="psb")
                nc.tensor.matmul(out=psb, lhsT=onesr[:1, :], rhs=scsh[:1, :],
                                 start=True, stop=True)
                scsh_b = bnpool.tile([P, 2 * FEAT], f32)
                nc.vector.tensor_copy(out=scsh_b, in_=psb)

                for ri, (rlo, rhi, _cols) in enumerate(rounds):
                    nf2 = (rhi - rlo) * FEAT
                    ob = orpool.tile([P, rt_max * FEAT], f32, tag="outr",
                                     name=f"obn{ri}")
                    nc.sync.dma_start(out=ob[:, :nf2],
                                      in_=out_pre[:, rlo * FEAT:rhi * FEAT])
                    o3 = ob[:, :nf2].rearrange("p (t f) -> p t f", f=FEAT)
                    nc.vector.tensor_tensor(
                        out=o3, in0=o3,
                        in1=_mid_bcast(scsh_b[:, :FEAT], rhi - rlo),
                        op=Alu.mult)
                    nc.vector.tensor_tensor(
                        out=o3, in0=o3,
                        in1=_mid_bcast(scsh_b[:, FEAT:], rhi - rlo),
                        op=Alu.add)
                    nc.sync.dma_start(out=out_d[:, rlo * FEAT:rhi * FEAT],
                                      in_=ob[:, :nf2])

    nc.compile()
    return nc


# ---------------------------------------------------------------- entrypoint
def kernel(h, norm, e, bn_weight, bn_bias, src, dst, **_ignored):
    global LAST_RESULTS
    from concourse import bass_utils

    h = np.ascontiguousarray(np.asarray(h, dtype=np.float32))
    norm = np.ascontiguousarray(np.asarray(norm, dtype=np.float32))
    bn_weight = np.ascontiguousarray(np.asarray(bn_weight, dtype=np.float32))
    bn_bias = np.ascontiguousarray(np.asarray(bn_bias, dtype=np.float32))
    src_i = np.asarray(src)
    dst_i = np.asarray(dst)
    assert h.shape == (N_NODES, FEAT) and src_i.shape == (N_EDGES,)

    key = (int(np.bitwise_xor.reduce(src_i.view(np.uint32))),
           int(np.bitwise_xor.reduce(dst_i.view(np.uint32))))
    if key in _CACHE:
        sched, nc = _CACHE[key]
    else:
        sched = _build_schedule(src_i, dst_i)
        nc = _build_program(sched)
        _CACHE[key] = (sched, nc)

    nt = sched["nt"]
    in_maps = []
    for c in range(NCORES):
        pc = sched["per_core"][c]
        # norm_own: pure reindexing of the input norm array (sharding)
        nown = norm[pc["ids"], 0].astype(np.float32)
        in_maps.append({
            "h_in": h,
            "norm_in": norm,
            "bnw_in": bn_weight,
            "bnb_in": bn_bias,
            "cidx_in": pc["cidx"],
            "ridx_in": pc["ridx"],
            "meta_in": pc["meta"],
            "nown_in": nown,
        })

    trace = bool(int(os.environ.get("KERNEL_TRACE", "0")))
    res = bass_utils.run_bass_kernel_spmd(
        nc, in_maps, core_ids=list(range(NCORES)), trace=trace)
    LAST_RESULTS = res

    out_full = np.empty((N_NODES, FEAT), np.float32)
    for c in range(NCORES):
        pc = sched["per_core"][c]
        arr = np.asarray(res.results[c]["out"]).reshape(P, nt, FEAT)
        vm = pc["vmask"]
        out_full[pc["ids"][vm]] = arr[vm]
    return out_full

